# revision 18
# baseline (speedup 1.0000x reference)
"""GemmaAttention (GQA, B=2 S=2048 HID=2048, 16 q-heads / 4 kv-heads, d=256)
on 8 Trainium2 NeuronCores.

Sharding: core = (batch b, head-group g) with b = core//4, g = core%4.
Each core computes q-heads [4g, 4g+4) and kv-head g (the reference's
repeat_kv quirk maps q-head h to kv-head h//4), producing a partial
o_proj output [S, HID] from its 1024 o_proj input features.  The host
sums the 4 partials per batch.  No collectives.

On-chip layout is "transposed" throughout: hsT [HID, S], qT/kT [d, S],
v natural [S, d], scores computed transposed [ks, qs] so that
 - softmax denominators come from a PE ones-matmul (partition reduce),
 - PV and o_proj need no on-chip transposes.
Softmax skips max-subtraction (score*scale is O(5), exp cannot overflow);
1/sqrt(d) is folded into exp's scale immediate; the additive mask is
pre-scaled by sqrt(d)=16 on the host and accumulated into the scores PSUM
via an identity matmul.
"""

import sys

sys.path.insert(0, "/opt/trn_rl_repo")

import math

import numpy as np
import ml_dtypes

import concourse.bacc as bacc
import concourse.bass as bass
import concourse.bass_isa as bass_isa
import concourse.tile as tile
from concourse import mybir
from concourse.bass_utils import run_bass_kernel_spmd

B, S, HID = 2, 2048, 2048
N_HEADS, N_KV, HEAD_DIM = 16, 4, 256
HD2 = HEAD_DIM // 2  # 128
ROPE_BASE = 10000.0
P = 128
QB = 512  # qs block width (moving free dim)
NSB = S // QB  # 4 s-blocks
NHT = HID // P  # 16 hidden chunks
NKS = S // P  # 16 key tiles
HPC = N_HEADS // 4  # 4 q heads per core
FQ = HPC * HEAD_DIM  # 1024 q features per core
NFQ = FQ // P  # 8 qT partition tiles
SCALE = 1.0 / math.sqrt(HEAD_DIM)

F32 = mybir.dt.float32
BF16 = mybir.dt.bfloat16
NP_BF16 = ml_dtypes.bfloat16


def _build(mask_mode: str, dt: "mybir.dt" = BF16):
    """mask_mode: 'causal' | 'none' | 'full'. Returns compiled Bacc."""
    nc = bacc.Bacc("TRN2", target_bir_lowering=False, debug=False, num_devices=8)

    hsT = nc.dram_tensor("hsT", [HID, S], dt, kind="ExternalInput").ap()
    wq = nc.dram_tensor("wq", [HID, FQ], dt, kind="ExternalInput").ap()
    wk = nc.dram_tensor("wk", [HID, HEAD_DIM], dt, kind="ExternalInput").ap()
    wv = nc.dram_tensor("wv", [HID, HEAD_DIM], dt, kind="ExternalInput").ap()
    wo = nc.dram_tensor("wo", [FQ, HID], dt, kind="ExternalInput").ap()
    cosT = nc.dram_tensor("cosT", [HD2, S], F32, kind="ExternalInput").ap()
    sinT = nc.dram_tensor("sinT", [HD2, S], F32, kind="ExternalInput").ap()
    if mask_mode == "causal":
        ident = nc.dram_tensor("ident", [P, P], dt, kind="ExternalInput").ap()
        mdiag = nc.dram_tensor("mdiag", [P, 896], dt, kind="ExternalInput").ap()
    elif mask_mode == "full":
        ident = nc.dram_tensor("ident", [P, P], dt, kind="ExternalInput").ap()
        maskT = nc.dram_tensor("maskT", [S, S], dt, kind="ExternalInput").ap()
    out = nc.dram_tensor("out", [S, HID], F32, kind="ExternalOutput").ap()

    with tile.TileContext(nc) as tc:
        with (
            tc.tile_pool(name="resid", bufs=1) as resid,
            tc.tile_pool(name="hst", bufs=1) as hst_pool,
            tc.tile_pool(name="oT", bufs=2) as oT_pool,
            tc.tile_pool(name="probs", bufs=3) as probs_pool,
            tc.tile_pool(name="tmp", bufs=1) as tmp_pool,
            tc.tile_pool(name="rb", bufs=1) as rb_pool,
            tc.tile_pool(name="mchunk", bufs=4) as mchunk_pool,
            tc.tile_pool(name="outsb", bufs=2) as outsb_pool,
            tc.tile_pool(name="wo", bufs=2) as wo_pool,
            tc.tile_pool(name="mm_ps", bufs=3, space="PSUM") as mm_ps,
            tc.tile_pool(name="sc_ps", bufs=3, space="PSUM") as sc_ps,
            tc.tile_pool(name="o_ps", bufs=2, space="PSUM") as o_ps,
            tc.tile_pool(name="sacc", bufs=2) as sacc_pool,
        ):
            # ---- persistent tiles ----
            qT = [resid.tile([P, S], dt, tag=f"qT{i}", name=f"qT{i}") for i in range(NFQ)]
            kT = [resid.tile([P, S], dt, tag=f"kT{i}", name=f"kT{i}") for i in range(2)]
            vt = [resid.tile([P, HEAD_DIM], dt, tag=f"v{i}", name=f"v{i}") for i in range(NKS)]
            cos_t = resid.tile([HD2, S], F32, tag="cos", name="cos_t")
            sin_t = resid.tile([HD2, S], F32, tag="sin", name="sin_t")
            wk_sl = resid.tile([P, NHT, HEAD_DIM], dt, tag="wk", name="wk_sl")
            wv_sl = resid.tile([P, NHT, HEAD_DIM], dt, tag="wv", name="wv_sl")
            wq_sl = resid.tile([P, NHT, FQ], dt, tag="wq", name="wq_sl")
            if mask_mode == "causal":
                id_t = resid.tile([P, P], dt, tag="ident", name="id_t")
                nc.sync.dma_start(out=id_t, in_=ident)
                md_t = resid.tile([P, 896], dt, tag="mdiag", name="md_t")
                nc.sync.dma_start(out=md_t, in_=mdiag)
            elif mask_mode == "full":
                id_t = resid.tile([P, P], dt, tag="ident", name="id_t")
                nc.sync.dma_start(out=id_t, in_=ident)

            def rope_pair(ps0, ps1, out0, out1, sb):
                """out0 = ps0*cos - ps1*sin ; out1 = ps1*cos + ps0*sin
                (tile column range sb*QB:+QB). Reads of ps0 first so its
                PSUM slot frees early."""
                cs = cos_t[:, sb * QB : (sb + 1) * QB]
                sn = sin_t[:, sb * QB : (sb + 1) * QB]
                t0 = tmp_pool.tile([P, QB], F32, tag="t0", name="t0")
                t1 = tmp_pool.tile([P, QB], F32, tag="t1", name="t1")
                t2 = tmp_pool.tile([P, QB], F32, tag="t2", name="t2")
                t3 = tmp_pool.tile([P, QB], F32, tag="t3", name="t3")
                nc.vector.tensor_mul(t0, ps0, cs)
                nc.vector.tensor_mul(t3, ps0, sn)
                nc.vector.tensor_mul(t1, ps1, sn)
                nc.vector.tensor_mul(t2, ps1, cs)
                sl = slice(sb * QB, (sb + 1) * QB)
                nc.vector.tensor_sub(out0[:, sl], t0, t1)
                nc.vector.tensor_add(out1[:, sl], t2, t3)

            # ================= phase 1: projections + rope =================
            for sb in range(NSB):
                ssl = slice(sb * QB, (sb + 1) * QB)
                hs_sl = hst_pool.tile([P, NHT, QB], dt, tag="hst", name="hs_sl")
                for hq in range(4):
                    hsl4 = slice(4 * hq, 4 * hq + 4)
                    if sb == 0:
                        nc.sync.dma_start(
                            out=wk_sl[:, hsl4, :],
                            in_=wk.rearrange("(t p) f -> p t f", p=P)[:, hsl4, :],
                        )
                    nc.sync.dma_start(
                        out=hs_sl[:, hsl4, :],
                        in_=hsT.rearrange("(t p) s -> p t s", p=P)[:, hsl4, ssl],
                    )
                if sb == 0:
                    for hq in range(4):
                        hsl4 = slice(4 * hq, 4 * hq + 4)
                        nc.sync.dma_start(
                            out=wv_sl[:, hsl4, :],
                            in_=wv.rearrange("(t p) f -> p t f", p=P)[:, hsl4, :],
                        )
                if sb == 0:
                    # deferred bulk loads: issue behind the first hsT slab so
                    # the first kT/v matmuls aren't queued behind 8MB of DMA
                    nc.sync.dma_start(out=cos_t, in_=cosT)
                    nc.sync.dma_start(out=sin_t, in_=sinT)
                    for hq in range(4):
                        nc.sync.dma_start(
                            out=wq_sl[:, 4 * hq : 4 * hq + 4, :],
                            in_=wq.rearrange("(t p) f -> p t f", p=P)[
                                :, 4 * hq : 4 * hq + 4, :
                            ],
                        )
                # k^T (one kv head: 2 d-halves), with rope
                ps_k = []
                for fd in range(2):
                    ps = mm_ps.tile([P, QB], F32, tag="mm", name="ps_mm")
                    for ht in range(NHT):
                        nc.tensor.matmul(
                            ps,
                            lhsT=wk_sl[:, ht, fd * P : (fd + 1) * P],
                            rhs=hs_sl[:, ht, :],
                            start=(ht == 0),
                            stop=(ht == NHT - 1),
                        )
                    ps_k.append(ps)
                rope_pair(ps_k[0], ps_k[1], kT[0], kT[1], sb)
                # v (natural layout [s, d])
                for s_sub in range(4):
                    ps = mm_ps.tile([P, QB], F32, tag="mm", name="ps_mm")
                    for ht in range(NHT):
                        nc.tensor.matmul(
                            ps[:, :HEAD_DIM],
                            lhsT=hs_sl[:, ht, s_sub * P : (s_sub + 1) * P],
                            rhs=wv_sl[:, ht, :],
                            start=(ht == 0),
                            stop=(ht == NHT - 1),
                        )
                    nc.any.tensor_copy(out=vt[sb * 4 + s_sub], in_=ps[:, :HEAD_DIM])
                # q^T (4 heads x 2 d-halves), with rope
                for h in range(HPC):
                    ps_q = []
                    for fd in range(2):
                        ft = 2 * h + fd
                        ps = mm_ps.tile([P, QB], F32, tag="mm", name="ps_mm")
                        for ht in range(NHT):
                            nc.tensor.matmul(
                                ps,
                                lhsT=wq_sl[:, ht, ft * P : (ft + 1) * P],
                                rhs=hs_sl[:, ht, :],
                                start=(ht == 0),
                                stop=(ht == NHT - 1),
                            )
                        ps_q.append(ps)
                    rope_pair(ps_q[0], ps_q[1], qT[2 * h], qT[2 * h + 1], sb)

            # ============ phase 2+3: attention + o_proj, per qs-block ============
            for qb in range(NSB):
                qsl = slice(qb * QB, (qb + 1) * QB)
                nks = 4 * qb + 4 if mask_mode == "causal" else NKS
                oT_qb = [oT_pool.tile([P, QB], dt, tag=f"oT{f}", name=f"oT{f}") for f in range(NFQ)]
                for h in range(HPC):
                    ps_o0 = o_ps.tile([P, QB], F32, tag="o", name="ps_o")
                    ps_o1 = o_ps.tile([P, QB], F32, tag="o", name="ps_o")
                    acc = sacc_pool.tile([P, QB], F32, tag="acc", name="acc")
                    for ks in range(nks):
                        ksl = slice(ks * P, (ks + 1) * P)
                        need_mask = (mask_mode == "full") or (
                            mask_mode == "causal" and ks >= 4 * qb
                        )
                        ps_s = sc_ps.tile([P, QB], F32, tag="sc", name="ps_s")
                        nc.tensor.matmul(
                            ps_s,
                            lhsT=kT[0][:, ksl],
                            rhs=qT[2 * h][:, qsl],
                            start=True,
                            stop=False,
                        )
                        nc.tensor.matmul(
                            ps_s,
                            lhsT=kT[1][:, ksl],
                            rhs=qT[2 * h + 1][:, qsl],
                            start=False,
                            stop=not need_mask,
                        )
                        if need_mask:
                            if mask_mode == "causal":
                                m = ks - 4 * qb
                                mrhs = md_t[:, 384 - 128 * m : 896 - 128 * m]
                            else:
                                mc = mchunk_pool.tile([P, QB], dt, tag="mc", name="mc")
                                nc.sync.dma_start(out=mc, in_=maskT[ksl, qsl])
                                mrhs = mc
                            nc.tensor.matmul(
                                ps_s, lhsT=id_t, rhs=mrhs, start=False, stop=True
                            )
                        probs = probs_pool.tile([P, QB], dt, tag="pr", name="probs")
                        nc.scalar.activation(
                            probs, ps_s, mybir.ActivationFunctionType.Exp, scale=SCALE
                        )
                        nc.tensor.matmul(
                            ps_o0,
                            lhsT=vt[ks][:, :HD2],
                            rhs=probs,
                            start=(ks == 0),
                            stop=(ks == nks - 1),
                        )
                        nc.tensor.matmul(
                            ps_o1,
                            lhsT=vt[ks][:, HD2:],
                            rhs=probs,
                            start=(ks == 0),
                            stop=(ks == nks - 1),
                        )
                        if ks == 0:
                            nc.vector.tensor_copy(out=acc, in_=probs)
                        else:
                            nc.vector.tensor_add(acc, acc, probs)
                    # evacuate unnormalized o^T (ACT), free PSUM fast
                    nc.any.tensor_copy(out=oT_qb[2 * h], in_=ps_o0)
                    nc.any.tensor_copy(out=oT_qb[2 * h + 1], in_=ps_o1)
                    # 1/colsum: partition all-reduce (broadcasts too), recip
                    zb = rb_pool.tile([P, QB], F32, tag="zb", name="zb")
                    nc.gpsimd.partition_all_reduce(
                        zb, acc, channels=P, reduce_op=bass_isa.ReduceOp.add
                    )
                    rb = rb_pool.tile([P, QB], F32, tag="rb", name="rb")
                    nc.vector.reciprocal_approx_fast(rb, zb)
                    nc.vector.tensor_mul(oT_qb[2 * h], oT_qb[2 * h], rb)
                    nc.vector.tensor_mul(oT_qb[2 * h + 1], oT_qb[2 * h + 1], rb)
                # ---- o_proj for this qs-block ----
                for hc in range(NSB):
                    hsl = slice(hc * QB, (hc + 1) * QB)
                    wo_sl = wo_pool.tile([P, NFQ, QB], dt, tag="wo", name="wo_sl")
                    nc.sync.dma_start(
                        out=wo_sl,
                        in_=wo.rearrange("(t p) h -> p t h", p=P)[:, :, hsl],
                    )
                    for s_sub in range(4):
                        ps = mm_ps.tile([P, QB], F32, tag="mm", name="ps_mm")
                        for f in range(NFQ):
                            nc.tensor.matmul(
                                ps,
                                lhsT=oT_qb[f][:, s_sub * P : (s_sub + 1) * P],
                                rhs=wo_sl[:, f, :],
                                start=(f == 0),
                                stop=(f == NFQ - 1),
                            )
                        ot = outsb_pool.tile([P, QB], F32, tag="ot", name="ot")
                        nc.any.tensor_copy(out=ot, in_=ps)
                        nc.sync.dma_start(
                            out=out[qb * QB + s_sub * P : qb * QB + (s_sub + 1) * P, hsl],
                            in_=ot,
                        )
    nc.compile()
    return nc


_BUILD_CACHE: dict = {}


def _get_kernel(mask_mode: str):
    if mask_mode not in _BUILD_CACHE:
        _BUILD_CACHE[mask_mode] = _build(mask_mode)
    return _BUILD_CACHE[mask_mode]


def _rope_tables(position_ids_b: np.ndarray):
    """cos/sin half-tables, transposed: [HD2, S] float32."""
    inv_freq = (
        1.0 / (ROPE_BASE ** (np.arange(0, HEAD_DIM, 2, dtype=np.float32) / HEAD_DIM))
    ).astype(np.float32)
    freqs = position_ids_b.astype(np.float32)[:, None] * inv_freq[None, :]  # [S, HD2]
    return (
        np.ascontiguousarray(np.cos(freqs).astype(np.float32).T),
        np.ascontiguousarray(np.sin(freqs).astype(np.float32).T),
    )


def kernel(hidden_states, attention_mask, position_ids, Wq, Wk, Wv, Wo):
    hidden_states = np.asarray(hidden_states, dtype=np.float32)
    attention_mask = np.asarray(attention_mask, dtype=np.float32)
    position_ids = np.asarray(position_ids)
    Wq = np.asarray(Wq, dtype=np.float32)
    Wk = np.asarray(Wk, dtype=np.float32)
    Wv = np.asarray(Wv, dtype=np.float32)
    Wo = np.asarray(Wo, dtype=np.float32)

    # mask classification
    tri = np.tril(np.ones((S, S), dtype=bool))
    canonical = np.where(tri, np.float32(0.0), np.float32(-1e9))
    is_causal = all(
        np.array_equal(attention_mask[b, 0], canonical) for b in range(B)
    )
    if is_causal:
        mask_mode = "causal"
    elif not attention_mask.any():
        mask_mode = "none"
    else:
        mask_mode = "full"

    nc = _get_kernel(mask_mode)

    ident = np.eye(P, dtype=np.float32).astype(NP_BF16)
    if mask_mode == "causal":
        ii = np.arange(P)[:, None]
        cc = np.arange(896)[None, :]
        mdiag = np.where(cc >= ii + 384, np.float32(0.0), np.float32(-16e9)).astype(NP_BF16)

    in_maps = []
    for core in range(8):
        b, g = core // 4, core % 4
        m = {
            "hsT": np.ascontiguousarray(hidden_states[b].T).astype(NP_BF16),
            "wq": np.ascontiguousarray(
                Wq[:, g * FQ : (g + 1) * FQ]
            ).astype(NP_BF16),
            "wk": np.ascontiguousarray(
                Wk[:, g * HEAD_DIM : (g + 1) * HEAD_DIM]
            ).astype(NP_BF16),
            "wv": np.ascontiguousarray(
                Wv[:, g * HEAD_DIM : (g + 1) * HEAD_DIM]
            ).astype(NP_BF16),
            "wo": np.ascontiguousarray(Wo[g * FQ : (g + 1) * FQ, :]).astype(NP_BF16),
        }
        cosT, sinT = _rope_tables(position_ids[b])
        m["cosT"], m["sinT"] = cosT, sinT
        if mask_mode == "causal":
            m["ident"] = ident
            m["mdiag"] = mdiag
        elif mask_mode == "full":
            m["ident"] = ident
            m["maskT"] = np.ascontiguousarray(
                (attention_mask[b, 0].T * np.float32(16.0))
            ).astype(NP_BF16)
        in_maps.append(m)

    global _LAST_IN_MAPS
    _LAST_IN_MAPS = in_maps
    res = run_bass_kernel_spmd(nc, in_maps, list(range(8)))
    outs = [res.results[c]["out"].astype(np.float32) for c in range(8)]
    full = np.empty((B, S, HID), dtype=np.float32)
    for b in range(B):
        full[b] = outs[4 * b] + outs[4 * b + 1] + outs[4 * b + 2] + outs[4 * b + 3]
    return full


# revision 22
# speedup vs baseline: 1.0058x; 1.0058x over previous
"""GemmaAttention (GQA, B=2 S=2048 HID=2048, 16 q-heads / 4 kv-heads, d=256)
on 8 Trainium2 NeuronCores.

Sharding: core = (batch b, head-group g) with b = core//4, g = core%4.
Each core computes q-heads [4g, 4g+4) and kv-head g (the reference's
repeat_kv quirk maps q-head h to kv-head h//4), producing a partial
o_proj output [S, HID] from its 1024 o_proj input features.  The host
sums the 4 partials per batch.  No collectives.

On-chip layout is "transposed" throughout: hsT [HID, S], qT/kT [d, S],
v natural [S, d], scores computed transposed [ks, qs] so that
 - softmax denominators come from a PE ones-matmul (partition reduce),
 - PV and o_proj need no on-chip transposes.
Softmax skips max-subtraction (score*scale is O(5), exp cannot overflow);
1/sqrt(d) is folded into exp's scale immediate; the additive mask is
pre-scaled by sqrt(d)=16 on the host and accumulated into the scores PSUM
via an identity matmul.
"""

import sys

sys.path.insert(0, "/opt/trn_rl_repo")

import math

import numpy as np
import ml_dtypes

import concourse.bacc as bacc
import concourse.bass as bass
import concourse.bass_isa as bass_isa
import concourse.tile as tile
from concourse import mybir
from concourse.bass_utils import run_bass_kernel_spmd

B, S, HID = 2, 2048, 2048
N_HEADS, N_KV, HEAD_DIM = 16, 4, 256
HD2 = HEAD_DIM // 2  # 128
ROPE_BASE = 10000.0
P = 128
QB = 512  # qs block width (moving free dim)
NSB = S // QB  # 4 s-blocks
NHT = HID // P  # 16 hidden chunks
NKS = S // P  # 16 key tiles
HPC = N_HEADS // 4  # 4 q heads per core
FQ = HPC * HEAD_DIM  # 1024 q features per core
NFQ = FQ // P  # 8 qT partition tiles
SCALE = 1.0 / math.sqrt(HEAD_DIM)

F32 = mybir.dt.float32
BF16 = mybir.dt.bfloat16
NP_BF16 = ml_dtypes.bfloat16


def _build(mask_mode: str, dt: "mybir.dt" = BF16):
    """mask_mode: 'causal' | 'none' | 'full'. Returns compiled Bacc."""
    nc = bacc.Bacc("TRN2", target_bir_lowering=False, debug=False, num_devices=8)

    hsT = nc.dram_tensor("hsT", [HID, S], dt, kind="ExternalInput").ap()
    wq = nc.dram_tensor("wq", [HID, FQ], dt, kind="ExternalInput").ap()
    wk = nc.dram_tensor("wk", [HID, HEAD_DIM], dt, kind="ExternalInput").ap()
    wv = nc.dram_tensor("wv", [HID, HEAD_DIM], dt, kind="ExternalInput").ap()
    wo = nc.dram_tensor("wo", [FQ, HID], dt, kind="ExternalInput").ap()
    cosT = nc.dram_tensor("cosT", [HD2, S], F32, kind="ExternalInput").ap()
    sinT = nc.dram_tensor("sinT", [HD2, S], F32, kind="ExternalInput").ap()
    if mask_mode == "causal":
        ident = nc.dram_tensor("ident", [P, P], dt, kind="ExternalInput").ap()
        mdiag = nc.dram_tensor("mdiag", [P, 896], dt, kind="ExternalInput").ap()
    elif mask_mode == "full":
        ident = nc.dram_tensor("ident", [P, P], dt, kind="ExternalInput").ap()
        maskT = nc.dram_tensor("maskT", [S, S], dt, kind="ExternalInput").ap()
    out = nc.dram_tensor("out", [S, HID], F32, kind="ExternalOutput").ap()

    with tile.TileContext(nc) as tc:
        with (
            tc.tile_pool(name="resid", bufs=1) as resid,
            tc.tile_pool(name="hst", bufs=1) as hst_pool,
            tc.tile_pool(name="oT", bufs=2) as oT_pool,
            tc.tile_pool(name="probs", bufs=4) as probs_pool,
            tc.tile_pool(name="tmp", bufs=1) as tmp_pool,
            tc.tile_pool(name="rb", bufs=1) as rb_pool,
            tc.tile_pool(name="mchunk", bufs=4) as mchunk_pool,
            tc.tile_pool(name="outsb", bufs=2) as outsb_pool,
            tc.tile_pool(name="wo", bufs=2) as wo_pool,
            tc.tile_pool(name="mm_ps", bufs=3, space="PSUM") as mm_ps,
            tc.tile_pool(name="sc_ps", bufs=3, space="PSUM") as sc_ps,
            tc.tile_pool(name="o_ps", bufs=2, space="PSUM") as o_ps,
            tc.tile_pool(name="sacc", bufs=2) as sacc_pool,
        ):
            # ---- persistent tiles ----
            qT = [resid.tile([P, S], dt, tag=f"qT{i}", name=f"qT{i}") for i in range(NFQ)]
            kT = [resid.tile([P, S], dt, tag=f"kT{i}", name=f"kT{i}") for i in range(2)]
            vt = [resid.tile([P, HEAD_DIM], dt, tag=f"v{i}", name=f"v{i}") for i in range(NKS)]
            cos_t = resid.tile([HD2, S], F32, tag="cos", name="cos_t")
            sin_t = resid.tile([HD2, S], F32, tag="sin", name="sin_t")
            wk_sl = resid.tile([P, NHT, HEAD_DIM], dt, tag="wk", name="wk_sl")
            wv_sl = resid.tile([P, NHT, HEAD_DIM], dt, tag="wv", name="wv_sl")
            wq_sl = resid.tile([P, NHT, FQ], dt, tag="wq", name="wq_sl")
            if mask_mode == "causal":
                id_t = resid.tile([P, P], dt, tag="ident", name="id_t")
                nc.sync.dma_start(out=id_t, in_=ident)
                md_t = resid.tile([P, 896], dt, tag="mdiag", name="md_t")
                nc.sync.dma_start(out=md_t, in_=mdiag)
            elif mask_mode == "full":
                id_t = resid.tile([P, P], dt, tag="ident", name="id_t")
                nc.sync.dma_start(out=id_t, in_=ident)

            def rope_pair(ps0, ps1, out0, out1, sb):
                """out0 = ps0*cos - ps1*sin ; out1 = ps1*cos + ps0*sin
                (tile column range sb*QB:+QB). Reads of ps0 first so its
                PSUM slot frees early."""
                cs = cos_t[:, sb * QB : (sb + 1) * QB]
                sn = sin_t[:, sb * QB : (sb + 1) * QB]
                t0 = tmp_pool.tile([P, QB], F32, tag="t0", name="t0")
                t1 = tmp_pool.tile([P, QB], F32, tag="t1", name="t1")
                t2 = tmp_pool.tile([P, QB], F32, tag="t2", name="t2")
                t3 = tmp_pool.tile([P, QB], F32, tag="t3", name="t3")
                nc.vector.tensor_mul(t0, ps0, cs)
                nc.vector.tensor_mul(t3, ps0, sn)
                nc.vector.tensor_mul(t1, ps1, sn)
                nc.vector.tensor_mul(t2, ps1, cs)
                sl = slice(sb * QB, (sb + 1) * QB)
                nc.vector.tensor_sub(out0[:, sl], t0, t1)
                nc.vector.tensor_add(out1[:, sl], t2, t3)

            # ================= phase 1: projections + rope =================
            for sb in range(NSB):
                ssl = slice(sb * QB, (sb + 1) * QB)
                hs_sl = hst_pool.tile([P, NHT, QB], dt, tag="hst", name="hs_sl")
                for hq in range(4):
                    hsl4 = slice(4 * hq, 4 * hq + 4)
                    if sb == 0:
                        nc.sync.dma_start(
                            out=wk_sl[:, hsl4, :],
                            in_=wk.rearrange("(t p) f -> p t f", p=P)[:, hsl4, :],
                        )
                    nc.sync.dma_start(
                        out=hs_sl[:, hsl4, :],
                        in_=hsT.rearrange("(t p) s -> p t s", p=P)[:, hsl4, ssl],
                    )
                if sb == 0:
                    for hq in range(4):
                        hsl4 = slice(4 * hq, 4 * hq + 4)
                        nc.sync.dma_start(
                            out=wv_sl[:, hsl4, :],
                            in_=wv.rearrange("(t p) f -> p t f", p=P)[:, hsl4, :],
                        )
                if sb == 0:
                    # deferred bulk loads: issue behind the first hsT slab so
                    # the first kT/v matmuls aren't queued behind 8MB of DMA
                    nc.sync.dma_start(out=cos_t, in_=cosT)
                    nc.sync.dma_start(out=sin_t, in_=sinT)
                    for hq in range(4):
                        nc.sync.dma_start(
                            out=wq_sl[:, 4 * hq : 4 * hq + 4, :],
                            in_=wq.rearrange("(t p) f -> p t f", p=P)[
                                :, 4 * hq : 4 * hq + 4, :
                            ],
                        )
                # k^T (one kv head: 2 d-halves), with rope
                ps_k = []
                for fd in range(2):
                    ps = mm_ps.tile([P, QB], F32, tag="mm", name="ps_mm")
                    for ht in range(NHT):
                        nc.tensor.matmul(
                            ps,
                            lhsT=wk_sl[:, ht, fd * P : (fd + 1) * P],
                            rhs=hs_sl[:, ht, :],
                            start=(ht == 0),
                            stop=(ht == NHT - 1),
                        )
                    ps_k.append(ps)
                rope_pair(ps_k[0], ps_k[1], kT[0], kT[1], sb)
                # v (natural layout [s, d])
                for s_sub in range(4):
                    ps = mm_ps.tile([P, QB], F32, tag="mm", name="ps_mm")
                    for ht in range(NHT):
                        nc.tensor.matmul(
                            ps[:, :HEAD_DIM],
                            lhsT=hs_sl[:, ht, s_sub * P : (s_sub + 1) * P],
                            rhs=wv_sl[:, ht, :],
                            start=(ht == 0),
                            stop=(ht == NHT - 1),
                        )
                    nc.any.tensor_copy(out=vt[sb * 4 + s_sub], in_=ps[:, :HEAD_DIM])
                # q^T (4 heads x 2 d-halves), with rope
                for h in range(HPC):
                    ps_q = []
                    for fd in range(2):
                        ft = 2 * h + fd
                        ps = mm_ps.tile([P, QB], F32, tag="mm", name="ps_mm")
                        for ht in range(NHT):
                            nc.tensor.matmul(
                                ps,
                                lhsT=wq_sl[:, ht, ft * P : (ft + 1) * P],
                                rhs=hs_sl[:, ht, :],
                                start=(ht == 0),
                                stop=(ht == NHT - 1),
                            )
                        ps_q.append(ps)
                    rope_pair(ps_q[0], ps_q[1], qT[2 * h], qT[2 * h + 1], sb)

            # ============ phase 2+3: attention + o_proj, per qs-block ============
            for qb in range(NSB):
                qsl = slice(qb * QB, (qb + 1) * QB)
                nks = 4 * qb + 4 if mask_mode == "causal" else NKS
                oT_qb = [oT_pool.tile([P, QB], dt, tag=f"oT{f}", name=f"oT{f}") for f in range(NFQ)]
                for h in range(HPC):
                    ps_o0 = o_ps.tile([P, QB], F32, tag="o", name="ps_o")
                    ps_o1 = o_ps.tile([P, QB], F32, tag="o", name="ps_o")
                    acc = sacc_pool.tile([P, QB], F32, tag="acc", name="acc")
                    for ks in range(nks):
                        ksl = slice(ks * P, (ks + 1) * P)
                        need_mask = (mask_mode == "full") or (
                            mask_mode == "causal" and ks >= 4 * qb
                        )
                        ps_s = sc_ps.tile([P, QB], F32, tag="sc", name="ps_s")
                        nc.tensor.matmul(
                            ps_s,
                            lhsT=kT[0][:, ksl],
                            rhs=qT[2 * h][:, qsl],
                            start=True,
                            stop=False,
                        )
                        nc.tensor.matmul(
                            ps_s,
                            lhsT=kT[1][:, ksl],
                            rhs=qT[2 * h + 1][:, qsl],
                            start=False,
                            stop=not need_mask,
                        )
                        if need_mask:
                            if mask_mode == "causal":
                                m = ks - 4 * qb
                                mrhs = md_t[:, 384 - 128 * m : 896 - 128 * m]
                            else:
                                mc = mchunk_pool.tile([P, QB], dt, tag="mc", name="mc")
                                nc.sync.dma_start(out=mc, in_=maskT[ksl, qsl])
                                mrhs = mc
                            nc.tensor.matmul(
                                ps_s, lhsT=id_t, rhs=mrhs, start=False, stop=True
                            )
                        probs = probs_pool.tile([P, QB], dt, tag="pr", name="probs")
                        nc.scalar.activation(
                            probs, ps_s, mybir.ActivationFunctionType.Exp, scale=SCALE
                        )
                        nc.tensor.matmul(
                            ps_o0,
                            lhsT=vt[ks][:, :HD2],
                            rhs=probs,
                            start=(ks == 0),
                            stop=(ks == nks - 1),
                        )
                        nc.tensor.matmul(
                            ps_o1,
                            lhsT=vt[ks][:, HD2:],
                            rhs=probs,
                            start=(ks == 0),
                            stop=(ks == nks - 1),
                        )
                        if ks == 0:
                            nc.vector.tensor_copy(out=acc, in_=probs)
                        else:
                            nc.vector.tensor_add(acc, acc, probs)
                    # evacuate unnormalized o^T (ACT), free PSUM fast
                    nc.any.tensor_copy(out=oT_qb[2 * h], in_=ps_o0)
                    nc.any.tensor_copy(out=oT_qb[2 * h + 1], in_=ps_o1)
                    # 1/colsum: partition all-reduce (broadcasts too), recip
                    zb = rb_pool.tile([P, QB], F32, tag="zb", name="zb")
                    nc.gpsimd.partition_all_reduce(
                        zb, acc, channels=P, reduce_op=bass_isa.ReduceOp.add
                    )
                    rb = rb_pool.tile([P, QB], F32, tag="rb", name="rb")
                    nc.vector.reciprocal_approx_fast(rb, zb)
                    nc.vector.tensor_mul(oT_qb[2 * h], oT_qb[2 * h], rb)
                    nc.vector.tensor_mul(oT_qb[2 * h + 1], oT_qb[2 * h + 1], rb)
                # ---- o_proj for this qs-block ----
                for hc in range(NSB):
                    hsl = slice(hc * QB, (hc + 1) * QB)
                    wo_sl = wo_pool.tile([P, NFQ, QB], dt, tag="wo", name="wo_sl")
                    nc.sync.dma_start(
                        out=wo_sl,
                        in_=wo.rearrange("(t p) h -> p t h", p=P)[:, :, hsl],
                    )
                    for s_sub in range(4):
                        ps = mm_ps.tile([P, QB], F32, tag="mm", name="ps_mm")
                        for f in range(NFQ):
                            nc.tensor.matmul(
                                ps,
                                lhsT=oT_qb[f][:, s_sub * P : (s_sub + 1) * P],
                                rhs=wo_sl[:, f, :],
                                start=(f == 0),
                                stop=(f == NFQ - 1),
                            )
                        ot = outsb_pool.tile([P, QB], F32, tag="ot", name="ot")
                        nc.any.tensor_copy(out=ot, in_=ps)
                        nc.sync.dma_start(
                            out=out[qb * QB + s_sub * P : qb * QB + (s_sub + 1) * P, hsl],
                            in_=ot,
                        )
    nc.compile()
    return nc


_BUILD_CACHE: dict = {}


def _get_kernel(mask_mode: str):
    if mask_mode not in _BUILD_CACHE:
        _BUILD_CACHE[mask_mode] = _build(mask_mode)
    return _BUILD_CACHE[mask_mode]


def _rope_tables(position_ids_b: np.ndarray):
    """cos/sin half-tables, transposed: [HD2, S] float32."""
    inv_freq = (
        1.0 / (ROPE_BASE ** (np.arange(0, HEAD_DIM, 2, dtype=np.float32) / HEAD_DIM))
    ).astype(np.float32)
    freqs = position_ids_b.astype(np.float32)[:, None] * inv_freq[None, :]  # [S, HD2]
    return (
        np.ascontiguousarray(np.cos(freqs).astype(np.float32).T),
        np.ascontiguousarray(np.sin(freqs).astype(np.float32).T),
    )


def kernel(hidden_states, attention_mask, position_ids, Wq, Wk, Wv, Wo):
    hidden_states = np.asarray(hidden_states, dtype=np.float32)
    attention_mask = np.asarray(attention_mask, dtype=np.float32)
    position_ids = np.asarray(position_ids)
    Wq = np.asarray(Wq, dtype=np.float32)
    Wk = np.asarray(Wk, dtype=np.float32)
    Wv = np.asarray(Wv, dtype=np.float32)
    Wo = np.asarray(Wo, dtype=np.float32)

    # mask classification
    tri = np.tril(np.ones((S, S), dtype=bool))
    canonical = np.where(tri, np.float32(0.0), np.float32(-1e9))
    is_causal = all(
        np.array_equal(attention_mask[b, 0], canonical) for b in range(B)
    )
    if is_causal:
        mask_mode = "causal"
    elif not attention_mask.any():
        mask_mode = "none"
    else:
        mask_mode = "full"

    nc = _get_kernel(mask_mode)

    ident = np.eye(P, dtype=np.float32).astype(NP_BF16)
    if mask_mode == "causal":
        ii = np.arange(P)[:, None]
        cc = np.arange(896)[None, :]
        mdiag = np.where(cc >= ii + 384, np.float32(0.0), np.float32(-16e9)).astype(NP_BF16)

    in_maps = []
    for core in range(8):
        b, g = core // 4, core % 4
        m = {
            "hsT": np.ascontiguousarray(hidden_states[b].T).astype(NP_BF16),
            "wq": np.ascontiguousarray(
                Wq[:, g * FQ : (g + 1) * FQ]
            ).astype(NP_BF16),
            "wk": np.ascontiguousarray(
                Wk[:, g * HEAD_DIM : (g + 1) * HEAD_DIM]
            ).astype(NP_BF16),
            "wv": np.ascontiguousarray(
                Wv[:, g * HEAD_DIM : (g + 1) * HEAD_DIM]
            ).astype(NP_BF16),
            "wo": np.ascontiguousarray(Wo[g * FQ : (g + 1) * FQ, :]).astype(NP_BF16),
        }
        cosT, sinT = _rope_tables(position_ids[b])
        m["cosT"], m["sinT"] = cosT, sinT
        if mask_mode == "causal":
            m["ident"] = ident
            m["mdiag"] = mdiag
        elif mask_mode == "full":
            m["ident"] = ident
            m["maskT"] = np.ascontiguousarray(
                (attention_mask[b, 0].T * np.float32(16.0))
            ).astype(NP_BF16)
        in_maps.append(m)

    global _LAST_IN_MAPS
    _LAST_IN_MAPS = in_maps
    res = run_bass_kernel_spmd(nc, in_maps, list(range(8)))
    outs = [res.results[c]["out"].astype(np.float32) for c in range(8)]
    full = np.empty((B, S, HID), dtype=np.float32)
    for b in range(B):
        full[b] = outs[4 * b] + outs[4 * b + 1] + outs[4 * b + 2] + outs[4 * b + 3]
    return full


# revision 23
# speedup vs baseline: 1.0707x; 1.0645x over previous
"""GemmaAttention (GQA, B=2 S=2048 HID=2048, 16 q-heads / 4 kv-heads, d=256)
on 8 Trainium2 NeuronCores.

Sharding: core = (batch b, head-group g) with b = core//4, g = core%4.
Each core computes q-heads [4g, 4g+4) and kv-head g (the reference's
repeat_kv quirk maps q-head h to kv-head h//4), producing a partial
o_proj output [S, HID] from its 1024 o_proj input features.  The host
sums the 4 partials per batch.  No collectives.

On-chip layout is "transposed" throughout: hsT [HID, S], qT/kT [d, S],
v natural [S, d], scores computed transposed [ks, qs] so that
 - softmax denominators come from a PE ones-matmul (partition reduce),
 - PV and o_proj need no on-chip transposes.
Softmax skips max-subtraction (score*scale is O(5), exp cannot overflow);
1/sqrt(d) is folded into exp's scale immediate; the additive mask is
pre-scaled by sqrt(d)=16 on the host and accumulated into the scores PSUM
via an identity matmul.
"""

import sys

sys.path.insert(0, "/opt/trn_rl_repo")

import math

import numpy as np
import ml_dtypes

import concourse.bacc as bacc
import concourse.bass as bass
import concourse.bass_isa as bass_isa
import concourse.tile as tile
from concourse import mybir
from concourse.bass_utils import run_bass_kernel_spmd

B, S, HID = 2, 2048, 2048
N_HEADS, N_KV, HEAD_DIM = 16, 4, 256
HD2 = HEAD_DIM // 2  # 128
ROPE_BASE = 10000.0
P = 128
QB = 512  # qs block width (moving free dim)
NSB = S // QB  # 4 s-blocks
NHT = HID // P  # 16 hidden chunks
NKS = S // P  # 16 key tiles
HPC = N_HEADS // 4  # 4 q heads per core
FQ = HPC * HEAD_DIM  # 1024 q features per core
NFQ = FQ // P  # 8 qT partition tiles
SCALE = 1.0 / math.sqrt(HEAD_DIM)

F32 = mybir.dt.float32
BF16 = mybir.dt.bfloat16
NP_BF16 = ml_dtypes.bfloat16


def _build(mask_mode: str, dt: "mybir.dt" = BF16):
    """mask_mode: 'causal' | 'none' | 'full'. Returns compiled Bacc."""
    nc = bacc.Bacc("TRN2", target_bir_lowering=False, debug=False, num_devices=8)

    hsT = nc.dram_tensor("hsT", [HID, S], dt, kind="ExternalInput").ap()
    wq = nc.dram_tensor("wq", [HID, FQ], dt, kind="ExternalInput").ap()
    wk = nc.dram_tensor("wk", [HID, HEAD_DIM], dt, kind="ExternalInput").ap()
    wv = nc.dram_tensor("wv", [HID, HEAD_DIM], dt, kind="ExternalInput").ap()
    wo = nc.dram_tensor("wo", [FQ, HID], dt, kind="ExternalInput").ap()
    cosT = nc.dram_tensor("cosT", [HD2, S], F32, kind="ExternalInput").ap()
    sinT = nc.dram_tensor("sinT", [HD2, S], F32, kind="ExternalInput").ap()
    if mask_mode == "causal":
        ident = nc.dram_tensor("ident", [P, P], dt, kind="ExternalInput").ap()
        mtri = nc.dram_tensor("mtri", [P, P], dt, kind="ExternalInput").ap()
    elif mask_mode == "full":
        ident = nc.dram_tensor("ident", [P, P], dt, kind="ExternalInput").ap()
        maskT = nc.dram_tensor("maskT", [S, S], dt, kind="ExternalInput").ap()
    out = nc.dram_tensor("out", [S, HID], F32, kind="ExternalOutput").ap()

    with tile.TileContext(nc) as tc:
        with (
            tc.tile_pool(name="resid", bufs=1) as resid,
            tc.tile_pool(name="hst", bufs=1) as hst_pool,
            tc.tile_pool(name="oT", bufs=2) as oT_pool,
            tc.tile_pool(name="probs", bufs=4) as probs_pool,
            tc.tile_pool(name="tmp", bufs=1) as tmp_pool,
            tc.tile_pool(name="rb", bufs=1) as rb_pool,
            tc.tile_pool(name="mchunk", bufs=4) as mchunk_pool,
            tc.tile_pool(name="outsb", bufs=2) as outsb_pool,
            tc.tile_pool(name="wo", bufs=2) as wo_pool,
            tc.tile_pool(name="mm_ps", bufs=3, space="PSUM") as mm_ps,
            tc.tile_pool(name="sc_ps", bufs=3, space="PSUM") as sc_ps,
            tc.tile_pool(name="o_ps", bufs=2, space="PSUM") as o_ps,
            tc.tile_pool(name="sacc", bufs=2) as sacc_pool,
        ):
            # ---- persistent tiles ----
            qT = [resid.tile([P, S], dt, tag=f"qT{i}", name=f"qT{i}") for i in range(NFQ)]
            kT = [resid.tile([P, S], dt, tag=f"kT{i}", name=f"kT{i}") for i in range(2)]
            vt = [resid.tile([P, HEAD_DIM], dt, tag=f"v{i}", name=f"v{i}") for i in range(NKS)]
            cos_t = resid.tile([HD2, S], F32, tag="cos", name="cos_t")
            sin_t = resid.tile([HD2, S], F32, tag="sin", name="sin_t")
            wk_sl = resid.tile([P, NHT, HEAD_DIM], dt, tag="wk", name="wk_sl")
            wv_sl = resid.tile([P, NHT, HEAD_DIM], dt, tag="wv", name="wv_sl")
            wq_sl = resid.tile([P, NHT, FQ], dt, tag="wq", name="wq_sl")
            if mask_mode == "causal":
                id_t = resid.tile([P, P], dt, tag="ident", name="id_t")
                nc.sync.dma_start(out=id_t, in_=ident)
                mt_t = resid.tile([P, P], dt, tag="mtri", name="mt_t")
                nc.sync.dma_start(out=mt_t, in_=mtri)
            elif mask_mode == "full":
                id_t = resid.tile([P, P], dt, tag="ident", name="id_t")
                nc.sync.dma_start(out=id_t, in_=ident)

            def rope_pair(ps0, ps1, out0, out1, sb):
                """out0 = ps0*cos - ps1*sin ; out1 = ps1*cos + ps0*sin
                (tile column range sb*QB:+QB). Reads of ps0 first so its
                PSUM slot frees early."""
                cs = cos_t[:, sb * QB : (sb + 1) * QB]
                sn = sin_t[:, sb * QB : (sb + 1) * QB]
                t0 = tmp_pool.tile([P, QB], F32, tag="t0", name="t0")
                t1 = tmp_pool.tile([P, QB], F32, tag="t1", name="t1")
                t2 = tmp_pool.tile([P, QB], F32, tag="t2", name="t2")
                t3 = tmp_pool.tile([P, QB], F32, tag="t3", name="t3")
                nc.vector.tensor_mul(t0, ps0, cs)
                nc.vector.tensor_mul(t3, ps0, sn)
                nc.vector.tensor_mul(t1, ps1, sn)
                nc.vector.tensor_mul(t2, ps1, cs)
                sl = slice(sb * QB, (sb + 1) * QB)
                nc.vector.tensor_sub(out0[:, sl], t0, t1)
                nc.vector.tensor_add(out1[:, sl], t2, t3)

            # ================= phase 1: projections + rope =================
            for sb in range(NSB):
                ssl = slice(sb * QB, (sb + 1) * QB)
                hs_sl = hst_pool.tile([P, NHT, QB], dt, tag="hst", name="hs_sl")
                for hq in range(4):
                    hsl4 = slice(4 * hq, 4 * hq + 4)
                    if sb == 0:
                        nc.sync.dma_start(
                            out=wk_sl[:, hsl4, :],
                            in_=wk.rearrange("(t p) f -> p t f", p=P)[:, hsl4, :],
                        )
                    nc.sync.dma_start(
                        out=hs_sl[:, hsl4, :],
                        in_=hsT.rearrange("(t p) s -> p t s", p=P)[:, hsl4, ssl],
                    )
                if sb == 0:
                    for hq in range(4):
                        hsl4 = slice(4 * hq, 4 * hq + 4)
                        nc.sync.dma_start(
                            out=wv_sl[:, hsl4, :],
                            in_=wv.rearrange("(t p) f -> p t f", p=P)[:, hsl4, :],
                        )
                if sb == 0:
                    # deferred bulk loads: issue behind the first hsT slab so
                    # the first kT/v matmuls aren't queued behind 8MB of DMA
                    nc.sync.dma_start(out=cos_t, in_=cosT)
                    nc.sync.dma_start(out=sin_t, in_=sinT)
                    for hq in range(4):
                        nc.sync.dma_start(
                            out=wq_sl[:, 4 * hq : 4 * hq + 4, :],
                            in_=wq.rearrange("(t p) f -> p t f", p=P)[
                                :, 4 * hq : 4 * hq + 4, :
                            ],
                        )
                # k^T (one kv head: 2 d-halves), with rope
                ps_k = []
                for fd in range(2):
                    ps = mm_ps.tile([P, QB], F32, tag="mm", name="ps_mm")
                    for ht in range(NHT):
                        nc.tensor.matmul(
                            ps,
                            lhsT=wk_sl[:, ht, fd * P : (fd + 1) * P],
                            rhs=hs_sl[:, ht, :],
                            start=(ht == 0),
                            stop=(ht == NHT - 1),
                        )
                    ps_k.append(ps)
                rope_pair(ps_k[0], ps_k[1], kT[0], kT[1], sb)
                # v (natural layout [s, d])
                for s_sub in range(4):
                    ps = mm_ps.tile([P, QB], F32, tag="mm", name="ps_mm")
                    for ht in range(NHT):
                        nc.tensor.matmul(
                            ps[:, :HEAD_DIM],
                            lhsT=hs_sl[:, ht, s_sub * P : (s_sub + 1) * P],
                            rhs=wv_sl[:, ht, :],
                            start=(ht == 0),
                            stop=(ht == NHT - 1),
                        )
                    nc.any.tensor_copy(out=vt[sb * 4 + s_sub], in_=ps[:, :HEAD_DIM])
                # q^T (4 heads x 2 d-halves), with rope
                for h in range(HPC):
                    ps_q = []
                    for fd in range(2):
                        ft = 2 * h + fd
                        ps = mm_ps.tile([P, QB], F32, tag="mm", name="ps_mm")
                        for ht in range(NHT):
                            nc.tensor.matmul(
                                ps,
                                lhsT=wq_sl[:, ht, ft * P : (ft + 1) * P],
                                rhs=hs_sl[:, ht, :],
                                start=(ht == 0),
                                stop=(ht == NHT - 1),
                            )
                        ps_q.append(ps)
                    rope_pair(ps_q[0], ps_q[1], qT[2 * h], qT[2 * h + 1], sb)

            # ============ phase 2+3: attention + o_proj, per qs-block ============
            for qb in range(NSB):
                qsl = slice(qb * QB, (qb + 1) * QB)
                nks = 4 * qb + 4 if mask_mode == "causal" else NKS
                oT_qb = [oT_pool.tile([P, QB], dt, tag=f"oT{f}", name=f"oT{f}") for f in range(NFQ)]
                for h in range(HPC):
                    ps_o0 = o_ps.tile([P, QB], F32, tag="o", name="ps_o")
                    ps_o1 = o_ps.tile([P, QB], F32, tag="o", name="ps_o")
                    acc = sacc_pool.tile([P, QB], F32, tag="acc", name="acc")
                    for ks in range(nks):
                        ksl = slice(ks * P, (ks + 1) * P)
                        m = ks - 4 * qb if mask_mode == "causal" else -1
                        # columns [0, 128m) of a diagonal tile are fully
                        # masked -> compute only the live range [c0, QB)
                        c0 = 128 * m if m > 0 else 0
                        w = QB - c0
                        qslw = slice(qb * QB + c0, (qb + 1) * QB)
                        ps_s = sc_ps.tile([P, QB], F32, tag="sc", name="ps_s")
                        if m >= 0:
                            # triangle mask opens the accumulation group
                            # (start clears the bank; scores then overwrite
                            # the never-written columns, accumulate on the
                            # triangle ones)
                            nc.tensor.matmul(
                                ps_s[:, c0 : c0 + P],
                                lhsT=id_t,
                                rhs=mt_t,
                                start=True,
                                stop=False,
                            )
                        nc.tensor.matmul(
                            ps_s[:, c0:],
                            lhsT=kT[0][:, ksl],
                            rhs=qT[2 * h][:, qslw],
                            start=(m < 0),
                            stop=False,
                        )
                        nc.tensor.matmul(
                            ps_s[:, c0:],
                            lhsT=kT[1][:, ksl],
                            rhs=qT[2 * h + 1][:, qslw],
                            start=False,
                            stop=(mask_mode != "full"),
                        )
                        if mask_mode == "full":
                            mc = mchunk_pool.tile([P, QB], dt, tag="mc", name="mc")
                            nc.sync.dma_start(out=mc, in_=maskT[ksl, qsl])
                            nc.tensor.matmul(
                                ps_s, lhsT=id_t, rhs=mc, start=False, stop=True
                            )
                        probs = probs_pool.tile([P, QB], dt, tag="pr", name="probs")
                        nc.scalar.activation(
                            probs[:, :w],
                            ps_s[:, c0:],
                            mybir.ActivationFunctionType.Exp,
                            scale=SCALE,
                        )
                        nc.tensor.matmul(
                            ps_o0[:, c0:],
                            lhsT=vt[ks][:, :HD2],
                            rhs=probs[:, :w],
                            start=(ks == 0),
                            stop=(ks == nks - 1),
                        )
                        nc.tensor.matmul(
                            ps_o1[:, c0:],
                            lhsT=vt[ks][:, HD2:],
                            rhs=probs[:, :w],
                            start=(ks == 0),
                            stop=(ks == nks - 1),
                        )
                        if ks == 0:
                            nc.vector.tensor_copy(out=acc, in_=probs)
                        else:
                            nc.vector.tensor_add(
                                acc[:, c0:], acc[:, c0:], probs[:, :w]
                            )
                    # evacuate unnormalized o^T (ACT), free PSUM fast
                    nc.any.tensor_copy(out=oT_qb[2 * h], in_=ps_o0)
                    nc.any.tensor_copy(out=oT_qb[2 * h + 1], in_=ps_o1)
                    # 1/colsum: partition all-reduce (broadcasts too), recip
                    zb = rb_pool.tile([P, QB], F32, tag="zb", name="zb")
                    nc.gpsimd.partition_all_reduce(
                        zb, acc, channels=P, reduce_op=bass_isa.ReduceOp.add
                    )
                    rb = rb_pool.tile([P, QB], F32, tag="rb", name="rb")
                    nc.vector.reciprocal_approx_fast(rb, zb)
                    nc.vector.tensor_mul(oT_qb[2 * h], oT_qb[2 * h], rb)
                    nc.vector.tensor_mul(oT_qb[2 * h + 1], oT_qb[2 * h + 1], rb)
                # ---- o_proj for this qs-block ----
                for hc in range(NSB):
                    hsl = slice(hc * QB, (hc + 1) * QB)
                    wo_sl = wo_pool.tile([P, NFQ, QB], dt, tag="wo", name="wo_sl")
                    nc.sync.dma_start(
                        out=wo_sl,
                        in_=wo.rearrange("(t p) h -> p t h", p=P)[:, :, hsl],
                    )
                    for s_sub in range(4):
                        ps = mm_ps.tile([P, QB], F32, tag="mm", name="ps_mm")
                        for f in range(NFQ):
                            nc.tensor.matmul(
                                ps,
                                lhsT=oT_qb[f][:, s_sub * P : (s_sub + 1) * P],
                                rhs=wo_sl[:, f, :],
                                start=(f == 0),
                                stop=(f == NFQ - 1),
                            )
                        ot = outsb_pool.tile([P, QB], F32, tag="ot", name="ot")
                        nc.any.tensor_copy(out=ot, in_=ps)
                        nc.sync.dma_start(
                            out=out[qb * QB + s_sub * P : qb * QB + (s_sub + 1) * P, hsl],
                            in_=ot,
                        )
    nc.compile()
    return nc


_BUILD_CACHE: dict = {}


def _get_kernel(mask_mode: str):
    if mask_mode not in _BUILD_CACHE:
        _BUILD_CACHE[mask_mode] = _build(mask_mode)
    return _BUILD_CACHE[mask_mode]


def _rope_tables(position_ids_b: np.ndarray):
    """cos/sin half-tables, transposed: [HD2, S] float32."""
    inv_freq = (
        1.0 / (ROPE_BASE ** (np.arange(0, HEAD_DIM, 2, dtype=np.float32) / HEAD_DIM))
    ).astype(np.float32)
    freqs = position_ids_b.astype(np.float32)[:, None] * inv_freq[None, :]  # [S, HD2]
    return (
        np.ascontiguousarray(np.cos(freqs).astype(np.float32).T),
        np.ascontiguousarray(np.sin(freqs).astype(np.float32).T),
    )


def kernel(hidden_states, attention_mask, position_ids, Wq, Wk, Wv, Wo):
    hidden_states = np.asarray(hidden_states, dtype=np.float32)
    attention_mask = np.asarray(attention_mask, dtype=np.float32)
    position_ids = np.asarray(position_ids)
    Wq = np.asarray(Wq, dtype=np.float32)
    Wk = np.asarray(Wk, dtype=np.float32)
    Wv = np.asarray(Wv, dtype=np.float32)
    Wo = np.asarray(Wo, dtype=np.float32)

    # mask classification
    tri = np.tril(np.ones((S, S), dtype=bool))
    canonical = np.where(tri, np.float32(0.0), np.float32(-1e9))
    is_causal = all(
        np.array_equal(attention_mask[b, 0], canonical) for b in range(B)
    )
    if is_causal:
        mask_mode = "causal"
    elif not attention_mask.any():
        mask_mode = "none"
    else:
        mask_mode = "full"

    nc = _get_kernel(mask_mode)

    ident = np.eye(P, dtype=np.float32).astype(NP_BF16)
    if mask_mode == "causal":
        ii = np.arange(P)[:, None]
        jj = np.arange(P)[None, :]
        mtri = np.where(jj >= ii, np.float32(0.0), np.float32(-16e9)).astype(NP_BF16)

    in_maps = []
    for core in range(8):
        b, g = core // 4, core % 4
        m = {
            "hsT": np.ascontiguousarray(hidden_states[b].T).astype(NP_BF16),
            "wq": np.ascontiguousarray(
                Wq[:, g * FQ : (g + 1) * FQ]
            ).astype(NP_BF16),
            "wk": np.ascontiguousarray(
                Wk[:, g * HEAD_DIM : (g + 1) * HEAD_DIM]
            ).astype(NP_BF16),
            "wv": np.ascontiguousarray(
                Wv[:, g * HEAD_DIM : (g + 1) * HEAD_DIM]
            ).astype(NP_BF16),
            "wo": np.ascontiguousarray(Wo[g * FQ : (g + 1) * FQ, :]).astype(NP_BF16),
        }
        cosT, sinT = _rope_tables(position_ids[b])
        m["cosT"], m["sinT"] = cosT, sinT
        if mask_mode == "causal":
            m["ident"] = ident
            m["mtri"] = mtri
        elif mask_mode == "full":
            m["ident"] = ident
            m["maskT"] = np.ascontiguousarray(
                (attention_mask[b, 0].T * np.float32(16.0))
            ).astype(NP_BF16)
        in_maps.append(m)

    global _LAST_IN_MAPS
    _LAST_IN_MAPS = in_maps
    res = run_bass_kernel_spmd(nc, in_maps, list(range(8)))
    outs = [res.results[c]["out"].astype(np.float32) for c in range(8)]
    full = np.empty((B, S, HID), dtype=np.float32)
    for b in range(B):
        full[b] = outs[4 * b] + outs[4 * b + 1] + outs[4 * b + 2] + outs[4 * b + 3]
    return full


# revision 25
# speedup vs baseline: 1.0750x; 1.0040x over previous
"""GemmaAttention (GQA, B=2 S=2048 HID=2048, 16 q-heads / 4 kv-heads, d=256)
on 8 Trainium2 NeuronCores.

Sharding: core = (batch b, head-group g) with b = core//4, g = core%4.
Each core computes q-heads [4g, 4g+4) and kv-head g (the reference's
repeat_kv quirk maps q-head h to kv-head h//4), producing a partial
o_proj output [S, HID] from its 1024 o_proj input features.  The host
sums the 4 partials per batch.  No collectives.

On-chip layout is "transposed" throughout: hsT [HID, S], qT/kT [d, S],
v natural [S, d], scores computed transposed [ks, qs] so that
 - softmax denominators come from a PE ones-matmul (partition reduce),
 - PV and o_proj need no on-chip transposes.
Softmax skips max-subtraction (score*scale is O(5), exp cannot overflow);
1/sqrt(d) is folded into exp's scale immediate; the additive mask is
pre-scaled by sqrt(d)=16 on the host and accumulated into the scores PSUM
via an identity matmul.
"""

import sys

sys.path.insert(0, "/opt/trn_rl_repo")

import math

import numpy as np
import ml_dtypes

import concourse.bacc as bacc
import concourse.bass as bass
import concourse.bass_isa as bass_isa
import concourse.tile as tile
from concourse import mybir
from concourse.bass_utils import run_bass_kernel_spmd

B, S, HID = 2, 2048, 2048
N_HEADS, N_KV, HEAD_DIM = 16, 4, 256
HD2 = HEAD_DIM // 2  # 128
ROPE_BASE = 10000.0
P = 128
QB = 512  # qs block width (moving free dim)
NSB = S // QB  # 4 s-blocks
NHT = HID // P  # 16 hidden chunks
NKS = S // P  # 16 key tiles
HPC = N_HEADS // 4  # 4 q heads per core
FQ = HPC * HEAD_DIM  # 1024 q features per core
NFQ = FQ // P  # 8 qT partition tiles
SCALE = 1.0 / math.sqrt(HEAD_DIM)

F32 = mybir.dt.float32
BF16 = mybir.dt.bfloat16
NP_BF16 = ml_dtypes.bfloat16


def _build(mask_mode: str, dt: "mybir.dt" = BF16):
    """mask_mode: 'causal' | 'none' | 'full'. Returns compiled Bacc."""
    nc = bacc.Bacc("TRN2", target_bir_lowering=False, debug=False, num_devices=8)

    hsT = nc.dram_tensor("hsT", [HID, S], dt, kind="ExternalInput").ap()
    wq = nc.dram_tensor("wq", [HID, FQ], dt, kind="ExternalInput").ap()
    wk = nc.dram_tensor("wk", [HID, HEAD_DIM], dt, kind="ExternalInput").ap()
    wv = nc.dram_tensor("wv", [HID, HEAD_DIM], dt, kind="ExternalInput").ap()
    wo = nc.dram_tensor("wo", [FQ, HID], dt, kind="ExternalInput").ap()
    cosT = nc.dram_tensor("cosT", [HD2, S], F32, kind="ExternalInput").ap()
    sinT = nc.dram_tensor("sinT", [HD2, S], F32, kind="ExternalInput").ap()
    if mask_mode == "causal":
        ident = nc.dram_tensor("ident", [P, P], dt, kind="ExternalInput").ap()
        mtri = nc.dram_tensor("mtri", [P, P], dt, kind="ExternalInput").ap()
    elif mask_mode == "full":
        ident = nc.dram_tensor("ident", [P, P], dt, kind="ExternalInput").ap()
        maskT = nc.dram_tensor("maskT", [S, S], dt, kind="ExternalInput").ap()
    out = nc.dram_tensor("out", [S, HID], F32, kind="ExternalOutput").ap()

    with tile.TileContext(nc) as tc:
        with (
            tc.tile_pool(name="resid", bufs=1) as resid,
            tc.tile_pool(name="hst", bufs=1) as hst_pool,
            tc.tile_pool(name="oT", bufs=2) as oT_pool,
            tc.tile_pool(name="probs", bufs=6) as probs_pool,
            tc.tile_pool(name="tmp", bufs=1) as tmp_pool,
            tc.tile_pool(name="rb", bufs=1) as rb_pool,
            tc.tile_pool(name="mchunk", bufs=4) as mchunk_pool,
            tc.tile_pool(name="outsb", bufs=2) as outsb_pool,
            tc.tile_pool(name="wo", bufs=3) as wo_pool,
            tc.tile_pool(name="mm_ps", bufs=3, space="PSUM") as mm_ps,
            tc.tile_pool(name="sc_ps", bufs=3, space="PSUM") as sc_ps,
            tc.tile_pool(name="o_ps", bufs=2, space="PSUM") as o_ps,
            tc.tile_pool(name="sacc", bufs=2) as sacc_pool,
        ):
            # ---- persistent tiles ----
            qT = [resid.tile([P, S], dt, tag=f"qT{i}", name=f"qT{i}") for i in range(NFQ)]
            kT = [resid.tile([P, S], dt, tag=f"kT{i}", name=f"kT{i}") for i in range(2)]
            vt = [resid.tile([P, HEAD_DIM], dt, tag=f"v{i}", name=f"v{i}") for i in range(NKS)]
            cos_t = resid.tile([HD2, S], F32, tag="cos", name="cos_t")
            sin_t = resid.tile([HD2, S], F32, tag="sin", name="sin_t")
            wk_sl = resid.tile([P, NHT, HEAD_DIM], dt, tag="wk", name="wk_sl")
            wv_sl = resid.tile([P, NHT, HEAD_DIM], dt, tag="wv", name="wv_sl")
            wq_sl = resid.tile([P, NHT, FQ], dt, tag="wq", name="wq_sl")
            if mask_mode == "causal":
                id_t = resid.tile([P, P], dt, tag="ident", name="id_t")
                nc.sync.dma_start(out=id_t, in_=ident)
                mt_t = resid.tile([P, P], dt, tag="mtri", name="mt_t")
                nc.sync.dma_start(out=mt_t, in_=mtri)
            elif mask_mode == "full":
                id_t = resid.tile([P, P], dt, tag="ident", name="id_t")
                nc.sync.dma_start(out=id_t, in_=ident)

            def rope_pair(ps0, ps1, out0, out1, sb):
                """out0 = ps0*cos - ps1*sin ; out1 = ps1*cos + ps0*sin
                (tile column range sb*QB:+QB). Reads of ps0 first so its
                PSUM slot frees early."""
                cs = cos_t[:, sb * QB : (sb + 1) * QB]
                sn = sin_t[:, sb * QB : (sb + 1) * QB]
                t0 = tmp_pool.tile([P, QB], F32, tag="t0", name="t0")
                t1 = tmp_pool.tile([P, QB], F32, tag="t1", name="t1")
                t2 = tmp_pool.tile([P, QB], F32, tag="t2", name="t2")
                t3 = tmp_pool.tile([P, QB], F32, tag="t3", name="t3")
                nc.vector.tensor_mul(t0, ps0, cs)
                nc.vector.tensor_mul(t3, ps0, sn)
                nc.vector.tensor_mul(t1, ps1, sn)
                nc.vector.tensor_mul(t2, ps1, cs)
                sl = slice(sb * QB, (sb + 1) * QB)
                nc.vector.tensor_sub(out0[:, sl], t0, t1)
                nc.vector.tensor_add(out1[:, sl], t2, t3)

            # ================= phase 1: projections + rope =================
            for sb in range(NSB):
                ssl = slice(sb * QB, (sb + 1) * QB)
                hs_sl = hst_pool.tile([P, NHT, QB], dt, tag="hst", name="hs_sl")
                for hq in range(4):
                    hsl4 = slice(4 * hq, 4 * hq + 4)
                    if sb == 0:
                        nc.sync.dma_start(
                            out=wk_sl[:, hsl4, :],
                            in_=wk.rearrange("(t p) f -> p t f", p=P)[:, hsl4, :],
                        )
                    nc.sync.dma_start(
                        out=hs_sl[:, hsl4, :],
                        in_=hsT.rearrange("(t p) s -> p t s", p=P)[:, hsl4, ssl],
                    )
                if sb == 0:
                    for hq in range(4):
                        hsl4 = slice(4 * hq, 4 * hq + 4)
                        nc.sync.dma_start(
                            out=wv_sl[:, hsl4, :],
                            in_=wv.rearrange("(t p) f -> p t f", p=P)[:, hsl4, :],
                        )
                if sb == 0:
                    # deferred bulk loads: issue behind the first hsT slab so
                    # the first kT/v matmuls aren't queued behind 8MB of DMA
                    nc.sync.dma_start(out=cos_t, in_=cosT)
                    nc.sync.dma_start(out=sin_t, in_=sinT)
                    for hq in range(4):
                        nc.sync.dma_start(
                            out=wq_sl[:, 4 * hq : 4 * hq + 4, :],
                            in_=wq.rearrange("(t p) f -> p t f", p=P)[
                                :, 4 * hq : 4 * hq + 4, :
                            ],
                        )
                # k^T (one kv head: 2 d-halves), with rope
                ps_k = []
                for fd in range(2):
                    ps = mm_ps.tile([P, QB], F32, tag="mm", name="ps_mm")
                    for ht in range(NHT):
                        nc.tensor.matmul(
                            ps,
                            lhsT=wk_sl[:, ht, fd * P : (fd + 1) * P],
                            rhs=hs_sl[:, ht, :],
                            start=(ht == 0),
                            stop=(ht == NHT - 1),
                        )
                    ps_k.append(ps)
                rope_pair(ps_k[0], ps_k[1], kT[0], kT[1], sb)
                # v (natural layout [s, d])
                for s_sub in range(4):
                    ps = mm_ps.tile([P, QB], F32, tag="mm", name="ps_mm")
                    for ht in range(NHT):
                        nc.tensor.matmul(
                            ps[:, :HEAD_DIM],
                            lhsT=hs_sl[:, ht, s_sub * P : (s_sub + 1) * P],
                            rhs=wv_sl[:, ht, :],
                            start=(ht == 0),
                            stop=(ht == NHT - 1),
                        )
                    nc.any.tensor_copy(out=vt[sb * 4 + s_sub], in_=ps[:, :HEAD_DIM])
                # q^T (4 heads x 2 d-halves), with rope
                for h in range(HPC):
                    ps_q = []
                    for fd in range(2):
                        ft = 2 * h + fd
                        ps = mm_ps.tile([P, QB], F32, tag="mm", name="ps_mm")
                        for ht in range(NHT):
                            nc.tensor.matmul(
                                ps,
                                lhsT=wq_sl[:, ht, ft * P : (ft + 1) * P],
                                rhs=hs_sl[:, ht, :],
                                start=(ht == 0),
                                stop=(ht == NHT - 1),
                            )
                        ps_q.append(ps)
                    rope_pair(ps_q[0], ps_q[1], qT[2 * h], qT[2 * h + 1], sb)

            # ============ phase 2+3: attention + o_proj, per qs-block ============
            for qb in range(NSB):
                qsl = slice(qb * QB, (qb + 1) * QB)
                nks = 4 * qb + 4 if mask_mode == "causal" else NKS
                oT_qb = [oT_pool.tile([P, QB], dt, tag=f"oT{f}", name=f"oT{f}") for f in range(NFQ)]
                for h in range(HPC):
                    ps_o0 = o_ps.tile([P, QB], F32, tag="o", name="ps_o")
                    ps_o1 = o_ps.tile([P, QB], F32, tag="o", name="ps_o")
                    acc = sacc_pool.tile([P, QB], F32, tag="acc", name="acc")
                    for ks in range(nks):
                        ksl = slice(ks * P, (ks + 1) * P)
                        m = ks - 4 * qb if mask_mode == "causal" else -1
                        # columns [0, 128m) of a diagonal tile are fully
                        # masked -> compute only the live range [c0, QB)
                        c0 = 128 * m if m > 0 else 0
                        w = QB - c0
                        qslw = slice(qb * QB + c0, (qb + 1) * QB)
                        ps_s = sc_ps.tile([P, QB], F32, tag="sc", name="ps_s")
                        if m >= 0:
                            # triangle mask opens the accumulation group
                            # (start clears the bank; scores then overwrite
                            # the never-written columns, accumulate on the
                            # triangle ones)
                            nc.tensor.matmul(
                                ps_s[:, c0 : c0 + P],
                                lhsT=id_t,
                                rhs=mt_t,
                                start=True,
                                stop=False,
                            )
                        nc.tensor.matmul(
                            ps_s[:, c0:],
                            lhsT=kT[0][:, ksl],
                            rhs=qT[2 * h][:, qslw],
                            start=(m < 0),
                            stop=False,
                        )
                        nc.tensor.matmul(
                            ps_s[:, c0:],
                            lhsT=kT[1][:, ksl],
                            rhs=qT[2 * h + 1][:, qslw],
                            start=False,
                            stop=(mask_mode != "full"),
                        )
                        if mask_mode == "full":
                            mc = mchunk_pool.tile([P, QB], dt, tag="mc", name="mc")
                            nc.sync.dma_start(out=mc, in_=maskT[ksl, qsl])
                            nc.tensor.matmul(
                                ps_s, lhsT=id_t, rhs=mc, start=False, stop=True
                            )
                        probs = probs_pool.tile([P, QB], dt, tag="pr", name="probs")
                        nc.scalar.activation(
                            probs[:, :w],
                            ps_s[:, c0:],
                            mybir.ActivationFunctionType.Exp,
                            scale=SCALE,
                        )
                        nc.tensor.matmul(
                            ps_o0[:, c0:],
                            lhsT=vt[ks][:, :HD2],
                            rhs=probs[:, :w],
                            start=(ks == 0),
                            stop=(ks == nks - 1),
                        )
                        nc.tensor.matmul(
                            ps_o1[:, c0:],
                            lhsT=vt[ks][:, HD2:],
                            rhs=probs[:, :w],
                            start=(ks == 0),
                            stop=(ks == nks - 1),
                        )
                        if ks == 0:
                            nc.vector.tensor_copy(out=acc, in_=probs)
                        else:
                            nc.vector.tensor_add(
                                acc[:, c0:], acc[:, c0:], probs[:, :w]
                            )
                    # evacuate unnormalized o^T (ACT), free PSUM fast
                    nc.any.tensor_copy(out=oT_qb[2 * h], in_=ps_o0)
                    nc.any.tensor_copy(out=oT_qb[2 * h + 1], in_=ps_o1)
                    # 1/colsum: partition all-reduce (broadcasts too), recip
                    zb = rb_pool.tile([P, QB], F32, tag="zb", name="zb")
                    nc.gpsimd.partition_all_reduce(
                        zb, acc, channels=P, reduce_op=bass_isa.ReduceOp.add
                    )
                    rb = rb_pool.tile([P, QB], F32, tag="rb", name="rb")
                    nc.vector.reciprocal_approx_fast(rb, zb)
                    nc.vector.tensor_mul(oT_qb[2 * h], oT_qb[2 * h], rb)
                    nc.vector.tensor_mul(oT_qb[2 * h + 1], oT_qb[2 * h + 1], rb)
                # ---- o_proj for this qs-block ----
                for hc in range(NSB):
                    hsl = slice(hc * QB, (hc + 1) * QB)
                    wo_sl = wo_pool.tile([P, NFQ, QB], dt, tag="wo", name="wo_sl")
                    nc.sync.dma_start(
                        out=wo_sl,
                        in_=wo.rearrange("(t p) h -> p t h", p=P)[:, :, hsl],
                    )
                    for s_sub in range(4):
                        ps = mm_ps.tile([P, QB], F32, tag="mm", name="ps_mm")
                        for f in range(NFQ):
                            nc.tensor.matmul(
                                ps,
                                lhsT=oT_qb[f][:, s_sub * P : (s_sub + 1) * P],
                                rhs=wo_sl[:, f, :],
                                start=(f == 0),
                                stop=(f == NFQ - 1),
                            )
                        ot = outsb_pool.tile([P, QB], F32, tag="ot", name="ot")
                        nc.any.tensor_copy(out=ot, in_=ps)
                        nc.sync.dma_start(
                            out=out[qb * QB + s_sub * P : qb * QB + (s_sub + 1) * P, hsl],
                            in_=ot,
                        )
    nc.compile()
    return nc


_BUILD_CACHE: dict = {}


def _get_kernel(mask_mode: str):
    if mask_mode not in _BUILD_CACHE:
        _BUILD_CACHE[mask_mode] = _build(mask_mode)
    return _BUILD_CACHE[mask_mode]


def _rope_tables(position_ids_b: np.ndarray):
    """cos/sin half-tables, transposed: [HD2, S] float32."""
    inv_freq = (
        1.0 / (ROPE_BASE ** (np.arange(0, HEAD_DIM, 2, dtype=np.float32) / HEAD_DIM))
    ).astype(np.float32)
    freqs = position_ids_b.astype(np.float32)[:, None] * inv_freq[None, :]  # [S, HD2]
    return (
        np.ascontiguousarray(np.cos(freqs).astype(np.float32).T),
        np.ascontiguousarray(np.sin(freqs).astype(np.float32).T),
    )


def kernel(hidden_states, attention_mask, position_ids, Wq, Wk, Wv, Wo):
    hidden_states = np.asarray(hidden_states, dtype=np.float32)
    attention_mask = np.asarray(attention_mask, dtype=np.float32)
    position_ids = np.asarray(position_ids)
    Wq = np.asarray(Wq, dtype=np.float32)
    Wk = np.asarray(Wk, dtype=np.float32)
    Wv = np.asarray(Wv, dtype=np.float32)
    Wo = np.asarray(Wo, dtype=np.float32)

    # mask classification
    tri = np.tril(np.ones((S, S), dtype=bool))
    canonical = np.where(tri, np.float32(0.0), np.float32(-1e9))
    is_causal = all(
        np.array_equal(attention_mask[b, 0], canonical) for b in range(B)
    )
    if is_causal:
        mask_mode = "causal"
    elif not attention_mask.any():
        mask_mode = "none"
    else:
        mask_mode = "full"

    nc = _get_kernel(mask_mode)

    ident = np.eye(P, dtype=np.float32).astype(NP_BF16)
    if mask_mode == "causal":
        ii = np.arange(P)[:, None]
        jj = np.arange(P)[None, :]
        mtri = np.where(jj >= ii, np.float32(0.0), np.float32(-16e9)).astype(NP_BF16)

    in_maps = []
    for core in range(8):
        b, g = core // 4, core % 4
        m = {
            "hsT": np.ascontiguousarray(hidden_states[b].T).astype(NP_BF16),
            "wq": np.ascontiguousarray(
                Wq[:, g * FQ : (g + 1) * FQ]
            ).astype(NP_BF16),
            "wk": np.ascontiguousarray(
                Wk[:, g * HEAD_DIM : (g + 1) * HEAD_DIM]
            ).astype(NP_BF16),
            "wv": np.ascontiguousarray(
                Wv[:, g * HEAD_DIM : (g + 1) * HEAD_DIM]
            ).astype(NP_BF16),
            "wo": np.ascontiguousarray(Wo[g * FQ : (g + 1) * FQ, :]).astype(NP_BF16),
        }
        cosT, sinT = _rope_tables(position_ids[b])
        m["cosT"], m["sinT"] = cosT, sinT
        if mask_mode == "causal":
            m["ident"] = ident
            m["mtri"] = mtri
        elif mask_mode == "full":
            m["ident"] = ident
            m["maskT"] = np.ascontiguousarray(
                (attention_mask[b, 0].T * np.float32(16.0))
            ).astype(NP_BF16)
        in_maps.append(m)

    global _LAST_IN_MAPS
    _LAST_IN_MAPS = in_maps
    res = run_bass_kernel_spmd(nc, in_maps, list(range(8)))
    outs = [res.results[c]["out"].astype(np.float32) for c in range(8)]
    full = np.empty((B, S, HID), dtype=np.float32)
    for b in range(B):
        full[b] = outs[4 * b] + outs[4 * b + 1] + outs[4 * b + 2] + outs[4 * b + 3]
    return full


# revision 27
# speedup vs baseline: 1.0841x; 1.0085x over previous
"""GemmaAttention (GQA, B=2 S=2048 HID=2048, 16 q-heads / 4 kv-heads, d=256)
on 8 Trainium2 NeuronCores.

Sharding: core = (batch b, head-group g) with b = core//4, g = core%4.
Each core computes q-heads [4g, 4g+4) and kv-head g (the reference's
repeat_kv quirk maps q-head h to kv-head h//4), producing a partial
o_proj output [S, HID] from its 1024 o_proj input features.  The host
sums the 4 partials per batch.  No collectives.

On-chip layout is "transposed" throughout: hsT [HID, S], qT/kT [d, S],
v natural [S, d], scores computed transposed [ks, qs] so that
 - softmax denominators come from a PE ones-matmul (partition reduce),
 - PV and o_proj need no on-chip transposes.
Softmax skips max-subtraction (score*scale is O(5), exp cannot overflow);
1/sqrt(d) is folded into exp's scale immediate; the additive mask is
pre-scaled by sqrt(d)=16 on the host and accumulated into the scores PSUM
via an identity matmul.  In the causal variant, diagonal ks-tiles are
column-sliced to their live range [128m, 512) — the triangle-mask matmul
(N=128) opens the PSUM accumulation group (start=True clears the bank;
the following score matmuls overwrite never-written columns and
accumulate onto the triangle), and scores/exp/PV/rowsum all run only on
the live columns.
"""

import sys

sys.path.insert(0, "/opt/trn_rl_repo")

import math

import numpy as np
import ml_dtypes

import concourse.bacc as bacc
import concourse.bass as bass
import concourse.bass_isa as bass_isa
import concourse.tile as tile
from concourse import mybir
from concourse.bass_utils import run_bass_kernel_spmd

B, S, HID = 2, 2048, 2048
N_HEADS, N_KV, HEAD_DIM = 16, 4, 256
HD2 = HEAD_DIM // 2  # 128
ROPE_BASE = 10000.0
P = 128
QB = 512  # qs block width (moving free dim)
NSB = S // QB  # 4 s-blocks
NHT = HID // P  # 16 hidden chunks
NKS = S // P  # 16 key tiles
HPC = N_HEADS // 4  # 4 q heads per core
FQ = HPC * HEAD_DIM  # 1024 q features per core
NFQ = FQ // P  # 8 qT partition tiles
SCALE = 1.0 / math.sqrt(HEAD_DIM)

F32 = mybir.dt.float32
BF16 = mybir.dt.bfloat16
NP_BF16 = ml_dtypes.bfloat16


def _build(mask_mode: str, dt: "mybir.dt" = BF16):
    """mask_mode: 'causal' | 'none' | 'full'. Returns compiled Bacc."""
    nc = bacc.Bacc("TRN2", target_bir_lowering=False, debug=False, num_devices=8)

    hsT = nc.dram_tensor("hsT", [HID, S], dt, kind="ExternalInput").ap()
    wq = nc.dram_tensor("wq", [HID, FQ], dt, kind="ExternalInput").ap()
    wk = nc.dram_tensor("wk", [HID, HEAD_DIM], dt, kind="ExternalInput").ap()
    wv = nc.dram_tensor("wv", [HID, HEAD_DIM], dt, kind="ExternalInput").ap()
    wo = nc.dram_tensor("wo", [FQ, HID], dt, kind="ExternalInput").ap()
    cosT = nc.dram_tensor("cosT", [HD2, S], F32, kind="ExternalInput").ap()
    sinT = nc.dram_tensor("sinT", [HD2, S], F32, kind="ExternalInput").ap()
    if mask_mode == "causal":
        ident = nc.dram_tensor("ident", [P, P], dt, kind="ExternalInput").ap()
        mtri = nc.dram_tensor("mtri", [P, P], dt, kind="ExternalInput").ap()
    elif mask_mode == "full":
        ident = nc.dram_tensor("ident", [P, P], dt, kind="ExternalInput").ap()
        maskT = nc.dram_tensor("maskT", [S, S], dt, kind="ExternalInput").ap()
    out = nc.dram_tensor("out", [S, HID], F32, kind="ExternalOutput").ap()

    with tile.TileContext(nc) as tc:
        with (
            tc.tile_pool(name="resid", bufs=1) as resid,
            tc.tile_pool(name="hst", bufs=1) as hst_pool,
            tc.tile_pool(name="oT", bufs=2) as oT_pool,
            tc.tile_pool(name="probs", bufs=6) as probs_pool,
            tc.tile_pool(name="tmp", bufs=1) as tmp_pool,
            tc.tile_pool(name="rb", bufs=1) as rb_pool,
            tc.tile_pool(name="mchunk", bufs=4) as mchunk_pool,
            tc.tile_pool(name="outsb", bufs=2) as outsb_pool,
            tc.tile_pool(name="wo", bufs=3) as wo_pool,
            tc.tile_pool(name="mm_ps", bufs=3, space="PSUM") as mm_ps,
            tc.tile_pool(name="sc_ps", bufs=3, space="PSUM") as sc_ps,
            tc.tile_pool(name="o_ps", bufs=2, space="PSUM") as o_ps,
            tc.tile_pool(name="sacc", bufs=2) as sacc_pool,
        ):
            # ---- persistent tiles ----
            qT = [resid.tile([P, S], dt, tag=f"qT{i}", name=f"qT{i}") for i in range(NFQ)]
            kT = [resid.tile([P, S], dt, tag=f"kT{i}", name=f"kT{i}") for i in range(2)]
            vt = [resid.tile([P, HEAD_DIM], dt, tag=f"v{i}", name=f"v{i}") for i in range(NKS)]
            cos_t = resid.tile([HD2, S], F32, tag="cos", name="cos_t")
            sin_t = resid.tile([HD2, S], F32, tag="sin", name="sin_t")
            wk_sl = resid.tile([P, NHT, HEAD_DIM], dt, tag="wk", name="wk_sl")
            wv_sl = resid.tile([P, NHT, HEAD_DIM], dt, tag="wv", name="wv_sl")
            wq_sl = resid.tile([P, NHT, FQ], dt, tag="wq", name="wq_sl")
            if mask_mode == "causal":
                id_t = resid.tile([P, P], dt, tag="ident", name="id_t")
                nc.sync.dma_start(out=id_t, in_=ident)
                mt_t = resid.tile([P, P], dt, tag="mtri", name="mt_t")
                nc.sync.dma_start(out=mt_t, in_=mtri)
            elif mask_mode == "full":
                id_t = resid.tile([P, P], dt, tag="ident", name="id_t")
                nc.sync.dma_start(out=id_t, in_=ident)

            def rope_pair(ps0, ps1, out0, out1, sb):
                """out0 = ps0*cos - ps1*sin ; out1 = ps1*cos + ps0*sin
                (tile column range sb*QB:+QB). Reads of ps0 first so its
                PSUM slot frees early."""
                cs = cos_t[:, sb * QB : (sb + 1) * QB]
                sn = sin_t[:, sb * QB : (sb + 1) * QB]
                t0 = tmp_pool.tile([P, QB], F32, tag="t0", name="t0")
                t1 = tmp_pool.tile([P, QB], F32, tag="t1", name="t1")
                t2 = tmp_pool.tile([P, QB], F32, tag="t2", name="t2")
                t3 = tmp_pool.tile([P, QB], F32, tag="t3", name="t3")
                nc.vector.tensor_mul(t0, ps0, cs)
                nc.vector.tensor_mul(t3, ps0, sn)
                nc.vector.tensor_mul(t1, ps1, sn)
                nc.vector.tensor_mul(t2, ps1, cs)
                sl = slice(sb * QB, (sb + 1) * QB)
                nc.vector.tensor_sub(out0[:, sl], t0, t1)
                nc.vector.tensor_add(out1[:, sl], t2, t3)

            # ================= phase 1: projections + rope =================
            for sb in range(NSB):
                ssl = slice(sb * QB, (sb + 1) * QB)
                hs_sl = hst_pool.tile([P, NHT, QB], dt, tag="hst", name="hs_sl")
                for hq in range(4):
                    hsl4 = slice(4 * hq, 4 * hq + 4)
                    if sb == 0:
                        nc.sync.dma_start(
                            out=wk_sl[:, hsl4, :],
                            in_=wk.rearrange("(t p) f -> p t f", p=P)[:, hsl4, :],
                        )
                    nc.sync.dma_start(
                        out=hs_sl[:, hsl4, :],
                        in_=hsT.rearrange("(t p) s -> p t s", p=P)[:, hsl4, ssl],
                    )
                # rope tables arrive as per-sb slices, keeping the startup
                # DMA window down to what the first kT/v matmuls need
                nc.sync.dma_start(out=cos_t[:, ssl], in_=cosT[:, ssl])
                nc.sync.dma_start(out=sin_t[:, ssl], in_=sinT[:, ssl])
                if sb == 0:
                    for hq in range(4):
                        hsl4 = slice(4 * hq, 4 * hq + 4)
                        nc.sync.dma_start(
                            out=wv_sl[:, hsl4, :],
                            in_=wv.rearrange("(t p) f -> p t f", p=P)[:, hsl4, :],
                        )
                    # deferred bulk load: behind the sb0 essentials
                    for hq in range(4):
                        nc.sync.dma_start(
                            out=wq_sl[:, 4 * hq : 4 * hq + 4, :],
                            in_=wq.rearrange("(t p) f -> p t f", p=P)[
                                :, 4 * hq : 4 * hq + 4, :
                            ],
                        )
                # k^T (one kv head: 2 d-halves), with rope
                ps_k = []
                for fd in range(2):
                    ps = mm_ps.tile([P, QB], F32, tag="mm", name="ps_mm")
                    for ht in range(NHT):
                        nc.tensor.matmul(
                            ps,
                            lhsT=wk_sl[:, ht, fd * P : (fd + 1) * P],
                            rhs=hs_sl[:, ht, :],
                            start=(ht == 0),
                            stop=(ht == NHT - 1),
                        )
                    ps_k.append(ps)
                rope_pair(ps_k[0], ps_k[1], kT[0], kT[1], sb)
                # v (natural layout [s, d])
                for s_sub in range(4):
                    ps = mm_ps.tile([P, QB], F32, tag="mm", name="ps_mm")
                    for ht in range(NHT):
                        nc.tensor.matmul(
                            ps[:, :HEAD_DIM],
                            lhsT=hs_sl[:, ht, s_sub * P : (s_sub + 1) * P],
                            rhs=wv_sl[:, ht, :],
                            start=(ht == 0),
                            stop=(ht == NHT - 1),
                        )
                    nc.any.tensor_copy(out=vt[sb * 4 + s_sub], in_=ps[:, :HEAD_DIM])
                # q^T (4 heads x 2 d-halves), with rope
                for h in range(HPC):
                    ps_q = []
                    for fd in range(2):
                        ft = 2 * h + fd
                        ps = mm_ps.tile([P, QB], F32, tag="mm", name="ps_mm")
                        for ht in range(NHT):
                            nc.tensor.matmul(
                                ps,
                                lhsT=wq_sl[:, ht, ft * P : (ft + 1) * P],
                                rhs=hs_sl[:, ht, :],
                                start=(ht == 0),
                                stop=(ht == NHT - 1),
                            )
                        ps_q.append(ps)
                    rope_pair(ps_q[0], ps_q[1], qT[2 * h], qT[2 * h + 1], sb)

            # ============ phase 2+3: attention + o_proj, per qs-block ============
            for qb in range(NSB):
                qsl = slice(qb * QB, (qb + 1) * QB)
                nks = 4 * qb + 4 if mask_mode == "causal" else NKS
                oT_qb = [oT_pool.tile([P, QB], dt, tag=f"oT{f}", name=f"oT{f}") for f in range(NFQ)]
                for h in range(HPC):
                    ps_o0 = o_ps.tile([P, QB], F32, tag="o", name="ps_o")
                    ps_o1 = o_ps.tile([P, QB], F32, tag="o", name="ps_o")
                    acc = sacc_pool.tile([P, QB], F32, tag="acc", name="acc")
                    for ks in range(nks):
                        ksl = slice(ks * P, (ks + 1) * P)
                        m = ks - 4 * qb if mask_mode == "causal" else -1
                        # columns [0, 128m) of a diagonal tile are fully
                        # masked -> compute only the live range [c0, QB)
                        c0 = 128 * m if m > 0 else 0
                        w = QB - c0
                        qslw = slice(qb * QB + c0, (qb + 1) * QB)
                        ps_s = sc_ps.tile([P, QB], F32, tag="sc", name="ps_s")
                        if m >= 0:
                            # triangle mask opens the accumulation group
                            # (start clears the bank; scores then overwrite
                            # the never-written columns, accumulate on the
                            # triangle ones)
                            nc.tensor.matmul(
                                ps_s[:, c0 : c0 + P],
                                lhsT=id_t,
                                rhs=mt_t,
                                start=True,
                                stop=False,
                            )
                        nc.tensor.matmul(
                            ps_s[:, c0:],
                            lhsT=kT[0][:, ksl],
                            rhs=qT[2 * h][:, qslw],
                            start=(m < 0),
                            stop=False,
                        )
                        nc.tensor.matmul(
                            ps_s[:, c0:],
                            lhsT=kT[1][:, ksl],
                            rhs=qT[2 * h + 1][:, qslw],
                            start=False,
                            stop=(mask_mode != "full"),
                        )
                        if mask_mode == "full":
                            mc = mchunk_pool.tile([P, QB], dt, tag="mc", name="mc")
                            nc.sync.dma_start(out=mc, in_=maskT[ksl, qsl])
                            nc.tensor.matmul(
                                ps_s, lhsT=id_t, rhs=mc, start=False, stop=True
                            )
                        probs = probs_pool.tile([P, QB], dt, tag="pr", name="probs")
                        nc.scalar.activation(
                            probs[:, :w],
                            ps_s[:, c0:],
                            mybir.ActivationFunctionType.Exp,
                            scale=SCALE,
                        )
                        nc.tensor.matmul(
                            ps_o0[:, c0:],
                            lhsT=vt[ks][:, :HD2],
                            rhs=probs[:, :w],
                            start=(ks == 0),
                            stop=(ks == nks - 1),
                        )
                        nc.tensor.matmul(
                            ps_o1[:, c0:],
                            lhsT=vt[ks][:, HD2:],
                            rhs=probs[:, :w],
                            start=(ks == 0),
                            stop=(ks == nks - 1),
                        )
                        if ks == 0:
                            nc.vector.tensor_copy(out=acc, in_=probs)
                        else:
                            nc.vector.tensor_add(
                                acc[:, c0:], acc[:, c0:], probs[:, :w]
                            )
                    # evacuate unnormalized o^T (ACT), free PSUM fast
                    nc.any.tensor_copy(out=oT_qb[2 * h], in_=ps_o0)
                    nc.any.tensor_copy(out=oT_qb[2 * h + 1], in_=ps_o1)
                    # 1/colsum: partition all-reduce (broadcasts too), recip
                    zb = rb_pool.tile([P, QB], F32, tag="zb", name="zb")
                    nc.gpsimd.partition_all_reduce(
                        zb, acc, channels=P, reduce_op=bass_isa.ReduceOp.add
                    )
                    rb = rb_pool.tile([P, QB], F32, tag="rb", name="rb")
                    nc.vector.reciprocal_approx_fast(rb, zb)
                    nc.vector.tensor_mul(oT_qb[2 * h], oT_qb[2 * h], rb)
                    nc.vector.tensor_mul(oT_qb[2 * h + 1], oT_qb[2 * h + 1], rb)
                # ---- o_proj for this qs-block ----
                for hc in range(NSB):
                    hsl = slice(hc * QB, (hc + 1) * QB)
                    wo_sl = wo_pool.tile([P, NFQ, QB], dt, tag="wo", name="wo_sl")
                    nc.sync.dma_start(
                        out=wo_sl,
                        in_=wo.rearrange("(t p) h -> p t h", p=P)[:, :, hsl],
                    )
                    for s_sub in range(4):
                        ps = mm_ps.tile([P, QB], F32, tag="mm", name="ps_mm")
                        for f in range(NFQ):
                            nc.tensor.matmul(
                                ps,
                                lhsT=oT_qb[f][:, s_sub * P : (s_sub + 1) * P],
                                rhs=wo_sl[:, f, :],
                                start=(f == 0),
                                stop=(f == NFQ - 1),
                            )
                        ot = outsb_pool.tile([P, QB], F32, tag="ot", name="ot")
                        nc.any.tensor_copy(out=ot, in_=ps)
                        nc.sync.dma_start(
                            out=out[qb * QB + s_sub * P : qb * QB + (s_sub + 1) * P, hsl],
                            in_=ot,
                        )
    nc.compile()
    return nc


_BUILD_CACHE: dict = {}


def _get_kernel(mask_mode: str):
    if mask_mode not in _BUILD_CACHE:
        _BUILD_CACHE[mask_mode] = _build(mask_mode)
    return _BUILD_CACHE[mask_mode]


def _rope_tables(position_ids_b: np.ndarray):
    """cos/sin half-tables, transposed: [HD2, S] float32."""
    inv_freq = (
        1.0 / (ROPE_BASE ** (np.arange(0, HEAD_DIM, 2, dtype=np.float32) / HEAD_DIM))
    ).astype(np.float32)
    freqs = position_ids_b.astype(np.float32)[:, None] * inv_freq[None, :]  # [S, HD2]
    return (
        np.ascontiguousarray(np.cos(freqs).astype(np.float32).T),
        np.ascontiguousarray(np.sin(freqs).astype(np.float32).T),
    )


def kernel(hidden_states, attention_mask, position_ids, Wq, Wk, Wv, Wo):
    hidden_states = np.asarray(hidden_states, dtype=np.float32)
    attention_mask = np.asarray(attention_mask, dtype=np.float32)
    position_ids = np.asarray(position_ids)
    Wq = np.asarray(Wq, dtype=np.float32)
    Wk = np.asarray(Wk, dtype=np.float32)
    Wv = np.asarray(Wv, dtype=np.float32)
    Wo = np.asarray(Wo, dtype=np.float32)

    # mask classification
    tri = np.tril(np.ones((S, S), dtype=bool))
    canonical = np.where(tri, np.float32(0.0), np.float32(-1e9))
    is_causal = all(
        np.array_equal(attention_mask[b, 0], canonical) for b in range(B)
    )
    if is_causal:
        mask_mode = "causal"
    elif not attention_mask.any():
        mask_mode = "none"
    else:
        mask_mode = "full"

    nc = _get_kernel(mask_mode)

    ident = np.eye(P, dtype=np.float32).astype(NP_BF16)
    if mask_mode == "causal":
        ii = np.arange(P)[:, None]
        jj = np.arange(P)[None, :]
        mtri = np.where(jj >= ii, np.float32(0.0), np.float32(-16e9)).astype(NP_BF16)

    in_maps = []
    for core in range(8):
        b, g = core // 4, core % 4
        m = {
            "hsT": np.ascontiguousarray(hidden_states[b].T).astype(NP_BF16),
            "wq": np.ascontiguousarray(
                Wq[:, g * FQ : (g + 1) * FQ]
            ).astype(NP_BF16),
            "wk": np.ascontiguousarray(
                Wk[:, g * HEAD_DIM : (g + 1) * HEAD_DIM]
            ).astype(NP_BF16),
            "wv": np.ascontiguousarray(
                Wv[:, g * HEAD_DIM : (g + 1) * HEAD_DIM]
            ).astype(NP_BF16),
            "wo": np.ascontiguousarray(Wo[g * FQ : (g + 1) * FQ, :]).astype(NP_BF16),
        }
        cosT, sinT = _rope_tables(position_ids[b])
        m["cosT"], m["sinT"] = cosT, sinT
        if mask_mode == "causal":
            m["ident"] = ident
            m["mtri"] = mtri
        elif mask_mode == "full":
            m["ident"] = ident
            m["maskT"] = np.ascontiguousarray(
                (attention_mask[b, 0].T * np.float32(16.0))
            ).astype(NP_BF16)
        in_maps.append(m)

    global _LAST_IN_MAPS
    _LAST_IN_MAPS = in_maps
    res = run_bass_kernel_spmd(nc, in_maps, list(range(8)))
    outs = [res.results[c]["out"].astype(np.float32) for c in range(8)]
    full = np.empty((B, S, HID), dtype=np.float32)
    for b in range(B):
        full[b] = outs[4 * b] + outs[4 * b + 1] + outs[4 * b + 2] + outs[4 * b + 3]
    return full
